# revision 10
# baseline (speedup 1.0000x reference)
"""Multi-head attention (B=4, S=1024, H=1024, 16 heads) on 8 TRN2 NeuronCores.

Sharding: tensor-parallel over heads - 2 heads per core. Each core computes
q/k/v projections for its 2 heads (full x replicated) and scores in
transposed [t, s] layout.

Softmax bias handling uses exp(s + b) = exp(s) * exp(b): the host
precomputes exp(bias^T) in bf16, the scalar engine computes exp(scores)
straight out of PSUM, and the vector engine multiplies by the bias factor
as a bf16*bf16 SBUF op (2x DVE mode) - no PE identity-matmul bias add and
no PSUM-bound vector add.  The softmax denominator rides along as a
ones-column appended to V; normalization happens on the host.

The emission schedule is software-pipelined: q/k projections for batch b+1
and the v projection for batch b are interleaved into the attention slots
of batch b so the tensor engine fills the gaps while the scalar engine
(exp) is the attention-phase bottleneck.  PV for head i runs during the
score slots of head i+1.

All DRAM operands are host-relaid so every DMA packet is a 2-4 KiB
contiguous run per partition.

Compute dtype bf16 (f32 PSUM accumulation); masks in this problem are all
False but are honored by folding -1e30 into the bias on host if ever set
(exp(-1e30) == 0 exactly on host).
"""

import numpy as np
import ml_dtypes

NUM_HEAD = 16
B, S, H = 4, 1024, 1024
HD = H // NUM_HEAD            # 64
N_CORES = 8
HPC = NUM_HEAD // N_CORES     # heads per core = 2
BH = B * HPC                  # batch-head pairs per core = 8
KO = H // 128                 # 8 contraction chunks
SC = S // 512                 # 2 (N=512 matmul chunks)
TT = S // 128                 # 8 (128-row tiles over s or t)
NHEADS = B * HPC              # 8 global head-slots per core

BF16 = ml_dtypes.bfloat16

_CACHE = {}

# Set by test harness to capture profiling info.
TRACE = False
LAST_RESULTS = None


def _build_bass():
    from concourse import bacc
    import concourse.tile as tile
    import concourse.mybir as mybir
    from contextlib import ExitStack

    bf16 = mybir.dt.bfloat16
    Exp = mybir.ActivationFunctionType.Exp
    Mult = mybir.AluOpType.mult

    nc = bacc.Bacc("TRN2", target_bir_lowering=False, debug=False)

    # host-relaid operands: per-partition lines are contiguous DRAM runs
    xt = nc.dram_tensor("xt", [B, 128, KO, S], bf16, kind="ExternalInput")
    wq = nc.dram_tensor("wq", [128, KO, 128], bf16, kind="ExternalInput")
    wk = nc.dram_tensor("wk", [128, KO, 128], bf16, kind="ExternalInput")
    wv = nc.dram_tensor("wv", [128, KO, 128], bf16, kind="ExternalInput")
    expb = nc.dram_tensor("expb", [BH, 128, TT, S], bf16, kind="ExternalInput")
    out = nc.dram_tensor("out", [BH, 128, TT * (HD + 1)], bf16, kind="ExternalOutput")

    with tile.TileContext(nc) as tc, ExitStack() as ctx:
        singles = ctx.enter_context(tc.tile_pool(name="singles", bufs=1))
        xpool = ctx.enter_context(tc.tile_pool(name="xpool", bufs=2))
        qtpool = ctx.enter_context(tc.tile_pool(name="qt", bufs=2))
        ktpool = ctx.enter_context(tc.tile_pool(name="kt", bufs=2))
        vpool = ctx.enter_context(tc.tile_pool(name="v", bufs=3))
        ptpool = ctx.enter_context(tc.tile_pool(name="pt", bufs=3))
        ebpool = ctx.enter_context(tc.tile_pool(name="eb", bufs=6))
        obhpool = ctx.enter_context(tc.tile_pool(name="obh", bufs=3))
        ps_proj = ctx.enter_context(tc.tile_pool(name="ps_proj", bufs=2, space="PSUM"))
        ps_scores = ctx.enter_context(
            tc.tile_pool(name="ps_scores", bufs=2, space="PSUM")
        )
        ps_out = ctx.enter_context(tc.tile_pool(name="ps_out", bufs=2, space="PSUM"))

        # PE warm-up: a dead-weight matmul chain on (uninitialized) SBUF
        # starts the moment the PE comes up, covering the p-state ramp while
        # the first DMAs land.  The result is discarded.
        warm = singles.tile([128, 512], bf16, tag="warm")
        wps = ps_proj.tile([128, 512], mybir.dt.float32, tag="ps_proj")
        for i in range(10):
            nc.tensor.matmul(
                wps[:], warm[:, 0:128], warm[:], start=(i == 0), stop=(i == 9)
            )
        nc.vector.tensor_copy(out=warm[:, 0:1], in_=wps[:, 0:1])

        # Startup-critical loads first: the very first matmul chain needs
        # only wq and xt[b=0] chunk 0 - everything else can trail.
        w_sb = {}
        for name in ("q", "k", "v"):
            w_sb[name] = singles.tile(
                [128, KO, 128], bf16, tag=f"w{name}", name=f"w{name}_sb"
            )
        nc.sync.dma_start(out=w_sb["q"][:], in_=wq[:])

        xt_sb = {}
        xt_sb[0] = xpool.tile([128, KO, S], bf16, tag="xt", name="xt0_sb")
        for kc in range(KO):
            nc.sync.dma_start(out=xt_sb[0][:, kc, :], in_=xt[0, :, kc, :])
            if kc == 0:
                nc.sync.dma_start(out=w_sb["k"][:], in_=wk[:])
            elif kc == 1:
                nc.sync.dma_start(out=w_sb["v"][:], in_=wv[:])

        qt_t = {}
        kt_t = {}
        vext_t = {}

        def gen_qk(b):
            """Yield (cols, fn) ops emitting batch b's q/k projections."""
            if b > 0:
                def dma_xt(b=b):
                    xt_sb[b] = xpool.tile(
                        [128, KO, S], bf16, tag="xt", name=f"xt{b}_sb"
                    )
                    for kc in range(0, KO, 2):
                        nc.sync.dma_start(
                            out=xt_sb[b][:, kc : kc + 2, :],
                            in_=xt[b, :, kc : kc + 2, :],
                        )
                yield (0, dma_xt)

            def alloc(b=b):
                qt_t[b] = qtpool.tile([128, S], bf16, tag="qt", name=f"qt{b}")
                kt_t[b] = ktpool.tile(
                    [128, HPC, S], bf16, tag="kt", name=f"kt{b}"
                )
                if b < 2:
                    # zero-pad halves persist across the 2-buf ring: only
                    # the first use of each buffer needs the memset.
                    nc.gpsimd.memset(kt_t[b][HD:128, 0, :], 0.0)
                    nc.gpsimd.memset(kt_t[b][0:HD, 1, :], 0.0)
            yield (0, alloc)

            # q then k: [j on partitions, s free]; per 512-chunk a psum chain
            # of 8 K-accumulation matmuls, cast to bf16 SBUF when done.
            for name in ("q", "k"):
                for sc in range(SC):
                    ssl = slice(sc * 512, (sc + 1) * 512)
                    chain = {}
                    for kop in range(0, KO, 2):
                        def mm_pair(
                            b=b, name=name, ssl=ssl, kop=kop, chain=chain
                        ):
                            if kop == 0:
                                chain["ps"] = ps_proj.tile(
                                    [128, 512], mybir.dt.float32,
                                    tag="ps_proj", name="ps_projc",
                                )
                            ps = chain["ps"]
                            for ko in (kop, kop + 1):
                                nc.tensor.matmul(
                                    ps[:],
                                    w_sb[name][:, ko, :],
                                    xt_sb[b][:, ko, ssl],
                                    start=(ko == 0),
                                    stop=(ko == KO - 1),
                                )
                            if kop + 2 == KO:
                                if name == "q":
                                    nc.scalar.copy(out=qt_t[b][:, ssl], in_=ps[:])
                                else:
                                    nc.vector.tensor_copy(
                                        out=kt_t[b][0:HD, 0, ssl], in_=ps[0:HD, :]
                                    )
                                    nc.vector.tensor_copy(
                                        out=kt_t[b][HD:128, 1, ssl],
                                        in_=ps[HD:128, :],
                                    )
                        yield (1024, mm_pair)

        def gen_v(b):
            """Yield (cols, fn) ops emitting batch b's v projection."""
            def alloc(b=b):
                vext_t[b] = vpool.tile(
                    [128, HPC, TT, HD + 1], bf16, tag="vext", name=f"vext{b}"
                )
                nc.gpsimd.memset(vext_t[b][:, :, :, HD : HD + 1], 1.0)
            yield (0, alloc)

        def gen_proj(b):
            yield from gen_qk(b)
            yield from gen_v(b)

            # v: [t on partitions, d free]; per 128-row t tile a psum chain,
            # one combined cast into both heads' vext slots.
            for tt in range(TT):
                tsl = slice(tt * 128, (tt + 1) * 128)
                chain = {}
                for kop in range(0, KO, 2):
                    def mm_pair_v(b=b, tsl=tsl, tt=tt, kop=kop, chain=chain):
                        if kop == 0:
                            chain["ps"] = ps_proj.tile(
                                [128, HPC, HD], mybir.dt.float32,
                                tag="ps_proj", name="ps_projv",
                            )
                        ps = chain["ps"]
                        for ko in (kop, kop + 1):
                            nc.tensor.matmul(
                                ps[:, :, :],
                                xt_sb[b][:, ko, tsl],
                                w_sb["v"][:, ko, :],
                                start=(ko == 0),
                                stop=(ko == KO - 1),
                            )
                        if kop + 2 == KO:
                            nc.vector.tensor_copy(
                                out=vext_t[b][:, :, tt, 0:HD], in_=ps[:, :, :]
                            )
                    yield (256, mm_pair_v)

        # Drain batch 0's projections upfront (nothing to overlap with yet).
        for _cols, fn in gen_proj(0):
            fn()

        TOTAL_SLOTS = NHEADS * TT  # 64
        eb_tiles = {}
        eb_cursor = 0

        def emit_eb_upto(limit):
            nonlocal eb_cursor
            while eb_cursor < min(limit, TOTAL_SLOTS):
                gi2, tt2 = divmod(eb_cursor, TT)
                ebt = ebpool.tile([128, S], bf16, tag="eb", name=f"eb{eb_cursor}")
                nc.sync.dma_start(out=ebt[:], in_=expb[gi2, :, tt2, :])
                eb_tiles[eb_cursor] = ebt
                eb_cursor += 1

        pt_t = {}
        obh_t = {}
        pso_group = {}

        def emit_pv_chunk(gi_prev, sc8):
            bp, hp = divmod(gi_prev, HPC)
            if sc8 == 0:
                obh_t[gi_prev] = obhpool.tile(
                    [128, TT, HD + 1], bf16, tag="obh", name=f"obh{gi_prev}"
                )
            if sc8 % 4 == 0:
                pso_group[0] = ps_out.tile(
                    [128, 4, HD + 1], mybir.dt.float32, tag="ps_out", name="pso_g"
                )
            pso = pso_group[0]
            ssl = slice(sc8 * 128, (sc8 + 1) * 128)
            ptp = pt_t[gi_prev]
            vxp = vext_t[bp]
            for ttp in range(TT):
                nc.tensor.matmul(
                    pso[:, sc8 % 4, :],
                    ptp[:, ttp, ssl],
                    vxp[:, hp, ttp, :],
                    start=(ttp == 0),
                    stop=(ttp == TT - 1),
                )
            if sc8 % 4 == 3:
                nc.vector.tensor_copy(
                    out=obh_t[gi_prev][:, sc8 - 3 : sc8 + 1, :], in_=pso[:, :, :]
                )
                half = slice((sc8 - 3) * (HD + 1), (sc8 + 1) * (HD + 1))
                nc.sync.dma_start(
                    out=out[gi_prev][:, half], in_=obh_t[gi_prev][:, sc8 - 3 : sc8 + 1, :]
                )

        proj_gen = None
        proj_total = proj_done = 0

        for gi in range(NHEADS):
            b, h = divmod(gi, HPC)
            if h == 0:
                # interleave batch b+1's projections over this batch's slots
                if b + 1 < B:
                    ops = list(gen_proj(b + 1))
                    proj_gen = iter(ops)
                    proj_total = sum(c for c, _ in ops)
                    proj_done = 0
                else:
                    proj_gen = None
            pt_t[gi] = ptpool.tile([128, TT, S], bf16, tag="pt", name=f"pt{gi}")

            for tt in range(TT):
                slot = gi * TT + tt
                tsl = slice(tt * 128, (tt + 1) * 128)
                emit_eb_upto(slot + 5)

                # scores: psum[t-tile, s] = ktz[h]^T . qt  (K=128,
                # zero-padded per head: K<128 matmuls stream ~2x slower)
                ps = ps_scores.tile([128, S], mybir.dt.float32, tag="ps_scores")
                for sc in range(SC):
                    ssl = slice(sc * 512, (sc + 1) * 512)
                    nc.tensor.matmul(
                        ps[:, ssl],
                        kt_t[b][:, h, tsl],
                        qt_t[b][:, ssl],
                        start=True,
                        stop=True,
                    )
                # exp on scalar engine, PSUM -> SBUF bf16
                nc.scalar.activation(out=pt_t[gi][:, tt, :], in_=ps[:], func=Exp)
                # fold in exp(bias) on vector engine (bf16 x bf16, in-place)
                nc.vector.tensor_tensor(
                    out=pt_t[gi][:, tt, :],
                    in0=pt_t[gi][:, tt, :],
                    in1=eb_tiles.pop(slot)[:],
                    op=Mult,
                )

                # PV chunk for the previous head
                if gi >= 1:
                    emit_pv_chunk(gi - 1, tt)

                # interleaved projection ops for batch b+1
                if proj_gen is not None:
                    k_in_batch = h * TT + tt
                    budget = (k_in_batch + 1) * proj_total / (HPC * TT)
                    while proj_done < budget:
                        try:
                            cols, fn = next(proj_gen)
                        except StopIteration:
                            proj_gen = None
                            break
                        fn()
                        proj_done += cols

        # tail: PV for the last head
        for sc8 in range(TT):
            emit_pv_chunk(NHEADS - 1, sc8)

    nc.compile()
    return nc


def kernel(x, attn_bias, attn_mask, padding_mask, Wq, Wk, Wv):
    global LAST_RESULTS
    from concourse.bass_utils import run_bass_kernel_spmd

    x = np.asarray(x, dtype=np.float32)
    attn_bias = np.asarray(attn_bias, dtype=np.float32)
    attn_mask = np.asarray(attn_mask)
    padding_mask = np.asarray(padding_mask)
    Wq = np.asarray(Wq, dtype=np.float32)
    Wk = np.asarray(Wk, dtype=np.float32)
    Wv = np.asarray(Wv, dtype=np.float32)

    scaling = HD ** -0.5
    # x^T per batch, partition-interleaved: [B, 128(p), KO, S]
    xt_full = np.ascontiguousarray(
        x.transpose(0, 2, 1).reshape(B, KO, 128, S).transpose(0, 2, 1, 3)
    ).astype(BF16)
    wqT = np.ascontiguousarray((Wq * scaling).T).astype(BF16)  # [k, j_global]
    wkT = np.ascontiguousarray(Wk.T).astype(BF16)
    wvT = np.ascontiguousarray(Wv.T).astype(BF16)

    bias_eff = attn_bias
    if attn_mask.any():
        bias_eff = bias_eff + np.where(attn_mask, -1e30, 0.0).astype(np.float32)[
            None, None
        ]
    if padding_mask.any():
        bias_eff = bias_eff + np.where(padding_mask, -1e30, 0.0).astype(np.float32)[
            :, None, None, :
        ]
    # [B, NH, t, s] so scores come out in transposed layout; exp() on host so
    # the kernel multiplies instead of adds (exp(-1e30) == 0 handles masks).
    expbT = np.exp(bias_eff.transpose(0, 1, 3, 2)).astype(BF16)
    # partition-interleave t: [B*NH, 128(p), TT, S]
    expb_r = np.ascontiguousarray(
        expbT.reshape(B * NUM_HEAD, TT, 128, S).transpose(0, 2, 1, 3)
    )

    def relay_w(w):  # [H(k), 128(j)] -> [128(p), KO, 128(j)]
        return np.ascontiguousarray(
            w.reshape(KO, 128, 128).transpose(1, 0, 2)
        )

    in_maps = []
    for c in range(N_CORES):
        jsl = slice(c * 128, (c + 1) * 128)
        # global head pairs for core c interleaved as [BH, ...]
        hsel = (
            np.arange(B)[:, None] * NUM_HEAD + (c * HPC + np.arange(HPC))[None, :]
        ).reshape(-1)
        in_maps.append(
            {
                "xt": xt_full,
                "wq": relay_w(wqT[:, jsl]),
                "wk": relay_w(wkT[:, jsl]),
                "wv": relay_w(wvT[:, jsl]),
                "expb": np.ascontiguousarray(expb_r[hsel]),
            }
        )

    if "nc" not in _CACHE:
        _CACHE["nc"] = _build_bass()
    nc = _CACHE["nc"]

    res = run_bass_kernel_spmd(
        nc, in_maps, core_ids=list(range(N_CORES)), trace=TRACE
    )
    LAST_RESULTS = res

    full = np.empty((B, S, H), np.float32)
    for c in range(N_CORES):
        oc = np.asarray(res.results[c]["out"]).astype(np.float32)
        oc = oc.reshape(BH, 128, TT, HD + 1)
        num = oc[..., :HD]
        den = oc[..., HD]
        o = num / den[..., None]                      # [BH, p, sc, d]
        o = o.transpose(0, 2, 1, 3).reshape(BH, S, HD)  # s = sc*128 + p
        full[:, :, c * 128 : (c + 1) * 128] = (
            o.reshape(B, HPC, S, HD).transpose(0, 2, 1, 3).reshape(B, S, HPC * HD)
        )
    return full


# revision 11
# speedup vs baseline: 1.0211x; 1.0211x over previous
"""Multi-head attention (B=4, S=1024, H=1024, 16 heads) on 8 TRN2 NeuronCores.

Sharding: batch x head-group - each core owns (one batch, 8 heads), i.e.
cores = 4 batches x 2 head-groups.  x^T for the core's batch is loaded once
(2.1 MB) and stays resident; projections run per head-pair (128 j columns)
so the schedule pipelines over 4 head-pairs exactly like a batch loop, but
with no per-batch x reload (saves 6.3 MB of DMA per core and removes the
xt burst that collided with the just-in-time exp(bias) stream).

Scores are computed in transposed [t, s] layout.  Softmax bias handling
uses exp(s + b) = exp(s) * exp(b): the host precomputes exp(bias^T) in
bf16, the scalar engine computes exp(scores) straight out of PSUM, and the
vector engine multiplies by the bias factor as a bf16*bf16 SBUF op (2x DVE
mode) - no PE identity-matmul bias add and no PSUM-bound vector add.  The
softmax denominator rides along as a ones-column appended to V;
normalization happens on the host.

The emission schedule is software-pipelined: projections for head-pair p+1
are interleaved into the attention slots of pair p so the tensor engine
fills the gaps while the scalar engine (exp) is the attention-phase
bottleneck.  PV for head i runs during the score slots of head i+1.

All DRAM operands are host-relaid so every DMA packet is a 2-4 KiB
contiguous run per partition.

Compute dtype bf16 (f32 PSUM accumulation); masks in this problem are all
False but are honored by folding -1e30 into the bias on host if ever set
(exp(-1e30) == 0 exactly on host).
"""

import numpy as np
import ml_dtypes

NUM_HEAD = 16
B, S, H = 4, 1024, 1024
HD = H // NUM_HEAD            # 64
N_CORES = 8
KO = H // 128                 # 8 contraction chunks
SC = S // 512                 # 2 (N=512 matmul chunks)
TT = S // 128                 # 8 (128-row tiles over s or t)
NP = 4                        # head-pairs per core
HPC = 2                       # heads per pair
NHEADS = NP * HPC             # 8 heads per core

BF16 = ml_dtypes.bfloat16

_CACHE = {}

# Set by test harness to capture profiling info.
TRACE = False
LAST_RESULTS = None


def _build_bass():
    from concourse import bacc
    import concourse.tile as tile
    import concourse.mybir as mybir
    from contextlib import ExitStack

    bf16 = mybir.dt.bfloat16
    Exp = mybir.ActivationFunctionType.Exp
    Mult = mybir.AluOpType.mult

    nc = bacc.Bacc("TRN2", target_bir_lowering=False, debug=False)

    # host-relaid operands: per-partition lines are contiguous DRAM runs
    xt = nc.dram_tensor("xt", [128, KO, S], bf16, kind="ExternalInput")
    wq = nc.dram_tensor("wq", [128, KO, NP, 128], bf16, kind="ExternalInput")
    wk = nc.dram_tensor("wk", [128, KO, NP, 128], bf16, kind="ExternalInput")
    wv = nc.dram_tensor("wv", [128, KO, NP, 128], bf16, kind="ExternalInput")
    expb = nc.dram_tensor("expb", [NHEADS, 128, TT, S], bf16, kind="ExternalInput")
    out = nc.dram_tensor(
        "out", [NHEADS, 128, TT * (HD + 1)], bf16, kind="ExternalOutput"
    )

    with tile.TileContext(nc) as tc, ExitStack() as ctx:
        singles = ctx.enter_context(tc.tile_pool(name="singles", bufs=1))
        qtpool = ctx.enter_context(tc.tile_pool(name="qt", bufs=2))
        ktpool = ctx.enter_context(tc.tile_pool(name="kt", bufs=2))
        vpool = ctx.enter_context(tc.tile_pool(name="v", bufs=3))
        ptpool = ctx.enter_context(tc.tile_pool(name="pt", bufs=3))
        ebpool = ctx.enter_context(tc.tile_pool(name="eb", bufs=6))
        obhpool = ctx.enter_context(tc.tile_pool(name="obh", bufs=3))
        ps_proj = ctx.enter_context(tc.tile_pool(name="ps_proj", bufs=2, space="PSUM"))
        ps_scores = ctx.enter_context(
            tc.tile_pool(name="ps_scores", bufs=2, space="PSUM")
        )
        ps_out = ctx.enter_context(tc.tile_pool(name="ps_out", bufs=2, space="PSUM"))

        # PE warm-up: a dead-weight matmul chain on (uninitialized) SBUF
        # starts the moment the PE comes up, covering the p-state ramp while
        # the first DMAs land.  The result is discarded.
        warm = singles.tile([128, 512], bf16, tag="warm")
        wps = ps_proj.tile([128, 512], mybir.dt.float32, tag="ps_proj")
        for i in range(10):
            nc.tensor.matmul(
                wps[:], warm[:, 0:128], warm[:], start=(i == 0), stop=(i == 9)
            )
        nc.vector.tensor_copy(out=warm[:, 0:1], in_=wps[:, 0:1])

        # Startup-critical loads: pair 0's weights and xt chunks first,
        # the trailing pairs' weights behind them.
        w_sb = {}
        w_dram = {}
        for name, dram in (("q", wq), ("k", wk), ("v", wv)):
            w_sb[name] = singles.tile(
                [128, KO, NP, 128], bf16, tag=f"w{name}", name=f"w{name}_sb"
            )
            w_dram[name] = dram
        nc.sync.dma_start(out=w_sb["q"][:, :, 0, :], in_=wq[:, :, 0, :])
        nc.sync.dma_start(out=w_sb["k"][:, :, 0, :], in_=wk[:, :, 0, :])

        xt_sb = singles.tile([128, KO, S], bf16, tag="xt", name="xt_sb")
        for kc in range(KO):
            nc.sync.dma_start(out=xt_sb[:, kc, :], in_=xt[:, kc, :])
            if kc == 0:
                nc.sync.dma_start(out=w_sb["v"][:, :, 0, :], in_=wv[:, :, 0, :])
        for name in ("q", "k", "v"):
            nc.sync.dma_start(
                out=w_sb[name][:, :, 1:NP, :], in_=w_dram[name][:, :, 1:NP, :]
            )

        qt_t = {}
        kt_t = {}
        vext_t = {}

        def gen_proj(p):
            """Yield (cols, fn) ops emitting head-pair p's projections."""
            def alloc(p=p):
                qt_t[p] = qtpool.tile([128, S], bf16, tag="qt", name=f"qt{p}")
                kt_t[p] = ktpool.tile(
                    [128, HPC, S], bf16, tag="kt", name=f"kt{p}"
                )
                if p < 2:
                    # zero-pad halves persist across the 2-buf ring: only
                    # the first use of each buffer needs the memset.
                    nc.gpsimd.memset(kt_t[p][HD:128, 0, :], 0.0)
                    nc.gpsimd.memset(kt_t[p][0:HD, 1, :], 0.0)
                vext_t[p] = vpool.tile(
                    [128, HPC, TT, HD + 1], bf16, tag="vext", name=f"vext{p}"
                )
                nc.gpsimd.memset(vext_t[p][:, :, :, HD : HD + 1], 1.0)
            yield (0, alloc)

            # q then k: [j on partitions, s free]; per 512-chunk a psum chain
            # of 8 K-accumulation matmuls, cast to bf16 SBUF when done.
            for name in ("q", "k"):
                for sc in range(SC):
                    ssl = slice(sc * 512, (sc + 1) * 512)
                    chain = {}
                    for kop in range(0, KO, 2):
                        def mm_pair(
                            p=p, name=name, ssl=ssl, kop=kop, chain=chain
                        ):
                            if kop == 0:
                                chain["ps"] = ps_proj.tile(
                                    [128, 512], mybir.dt.float32,
                                    tag="ps_proj", name="ps_projc",
                                )
                            ps = chain["ps"]
                            for ko in (kop, kop + 1):
                                nc.tensor.matmul(
                                    ps[:],
                                    w_sb[name][:, ko, p, :],
                                    xt_sb[:, ko, ssl],
                                    start=(ko == 0),
                                    stop=(ko == KO - 1),
                                )
                            if kop + 2 == KO:
                                if name == "q":
                                    nc.scalar.copy(out=qt_t[p][:, ssl], in_=ps[:])
                                else:
                                    nc.vector.tensor_copy(
                                        out=kt_t[p][0:HD, 0, ssl], in_=ps[0:HD, :]
                                    )
                                    nc.vector.tensor_copy(
                                        out=kt_t[p][HD:128, 1, ssl],
                                        in_=ps[HD:128, :],
                                    )
                        yield (1024, mm_pair)

            # v: [t on partitions, d free]; per 128-row t tile a psum chain,
            # one combined cast into both heads' vext slots.
            for tt in range(TT):
                tsl = slice(tt * 128, (tt + 1) * 128)
                chain = {}
                for kop in range(0, KO, 2):
                    def mm_pair_v(p=p, tsl=tsl, tt=tt, kop=kop, chain=chain):
                        if kop == 0:
                            chain["ps"] = ps_proj.tile(
                                [128, HPC, HD], mybir.dt.float32,
                                tag="ps_proj", name="ps_projv",
                            )
                        ps = chain["ps"]
                        for ko in (kop, kop + 1):
                            nc.tensor.matmul(
                                ps[:, :, :],
                                xt_sb[:, ko, tsl],
                                w_sb["v"][:, ko, p, :],
                                start=(ko == 0),
                                stop=(ko == KO - 1),
                            )
                        if kop + 2 == KO:
                            nc.vector.tensor_copy(
                                out=vext_t[p][:, :, tt, 0:HD], in_=ps[:, :, :]
                            )
                    yield (256, mm_pair_v)

        # Drain pair 0's projections upfront (nothing to overlap with yet).
        for _cols, fn in gen_proj(0):
            fn()

        TOTAL_SLOTS = NHEADS * TT  # 64
        eb_tiles = {}
        eb_cursor = 0

        def emit_eb_upto(limit):
            nonlocal eb_cursor
            while eb_cursor < min(limit, TOTAL_SLOTS):
                gi2, tt2 = divmod(eb_cursor, TT)
                ebt = ebpool.tile([128, S], bf16, tag="eb", name=f"eb{eb_cursor}")
                nc.sync.dma_start(out=ebt[:], in_=expb[gi2, :, tt2, :])
                eb_tiles[eb_cursor] = ebt
                eb_cursor += 1

        pt_t = {}
        obh_t = {}
        pso_group = {}

        def emit_pv_chunk(gi_prev, sc8):
            pp, hp = divmod(gi_prev, HPC)
            if sc8 == 0:
                obh_t[gi_prev] = obhpool.tile(
                    [128, TT, HD + 1], bf16, tag="obh", name=f"obh{gi_prev}"
                )
            if sc8 % 4 == 0:
                pso_group[0] = ps_out.tile(
                    [128, 4, HD + 1], mybir.dt.float32, tag="ps_out", name="pso_g"
                )
            pso = pso_group[0]
            ssl = slice(sc8 * 128, (sc8 + 1) * 128)
            ptp = pt_t[gi_prev]
            vxp = vext_t[pp]
            for ttp in range(TT):
                nc.tensor.matmul(
                    pso[:, sc8 % 4, :],
                    ptp[:, ttp, ssl],
                    vxp[:, hp, ttp, :],
                    start=(ttp == 0),
                    stop=(ttp == TT - 1),
                )
            if sc8 % 4 == 3:
                nc.vector.tensor_copy(
                    out=obh_t[gi_prev][:, sc8 - 3 : sc8 + 1, :], in_=pso[:, :, :]
                )
                half = slice((sc8 - 3) * (HD + 1), (sc8 + 1) * (HD + 1))
                nc.sync.dma_start(
                    out=out[gi_prev][:, half],
                    in_=obh_t[gi_prev][:, sc8 - 3 : sc8 + 1, :],
                )

        proj_gen = None
        proj_total = proj_done = 0

        for gi in range(NHEADS):
            p, h = divmod(gi, HPC)
            if h == 0:
                # interleave pair p+1's projections over this pair's slots
                if p + 1 < NP:
                    ops = list(gen_proj(p + 1))
                    proj_gen = iter(ops)
                    proj_total = sum(c for c, _ in ops)
                    proj_done = 0
                else:
                    proj_gen = None
            pt_t[gi] = ptpool.tile([128, TT, S], bf16, tag="pt", name=f"pt{gi}")

            for tt in range(TT):
                slot = gi * TT + tt
                tsl = slice(tt * 128, (tt + 1) * 128)
                emit_eb_upto(slot + 5)

                # scores: psum[t-tile, s] = ktz[h]^T . qt  (K=128,
                # zero-padded per head: K<128 matmuls stream ~2x slower)
                ps = ps_scores.tile([128, S], mybir.dt.float32, tag="ps_scores")
                for sc in range(SC):
                    ssl = slice(sc * 512, (sc + 1) * 512)
                    nc.tensor.matmul(
                        ps[:, ssl],
                        kt_t[p][:, h, tsl],
                        qt_t[p][:, ssl],
                        start=True,
                        stop=True,
                    )
                # exp on scalar engine, PSUM -> SBUF bf16
                nc.scalar.activation(out=pt_t[gi][:, tt, :], in_=ps[:], func=Exp)
                # fold in exp(bias) on vector engine (bf16 x bf16, in-place)
                nc.vector.tensor_tensor(
                    out=pt_t[gi][:, tt, :],
                    in0=pt_t[gi][:, tt, :],
                    in1=eb_tiles.pop(slot)[:],
                    op=Mult,
                )

                # PV chunk for the previous head
                if gi >= 1:
                    emit_pv_chunk(gi - 1, tt)

                # interleaved projection ops for pair p+1
                if proj_gen is not None:
                    k_in_pair = h * TT + tt
                    budget = (k_in_pair + 1) * proj_total / (HPC * TT)
                    while proj_done < budget:
                        try:
                            cols, fn = next(proj_gen)
                        except StopIteration:
                            proj_gen = None
                            break
                        fn()
                        proj_done += cols

        # tail: PV for the last head
        for sc8 in range(TT):
            emit_pv_chunk(NHEADS - 1, sc8)

    nc.compile()
    return nc


def kernel(x, attn_bias, attn_mask, padding_mask, Wq, Wk, Wv):
    global LAST_RESULTS
    from concourse.bass_utils import run_bass_kernel_spmd

    x = np.asarray(x, dtype=np.float32)
    attn_bias = np.asarray(attn_bias, dtype=np.float32)
    attn_mask = np.asarray(attn_mask)
    padding_mask = np.asarray(padding_mask)
    Wq = np.asarray(Wq, dtype=np.float32)
    Wk = np.asarray(Wk, dtype=np.float32)
    Wv = np.asarray(Wv, dtype=np.float32)

    scaling = HD ** -0.5
    # x^T per batch, partition-interleaved: [B, 128(p), KO, S]
    xt_full = np.ascontiguousarray(
        x.transpose(0, 2, 1).reshape(B, KO, 128, S).transpose(0, 2, 1, 3)
    ).astype(BF16)
    wqT = np.ascontiguousarray((Wq * scaling).T).astype(BF16)  # [k, j_global]
    wkT = np.ascontiguousarray(Wk.T).astype(BF16)
    wvT = np.ascontiguousarray(Wv.T).astype(BF16)

    bias_eff = attn_bias
    if attn_mask.any():
        bias_eff = bias_eff + np.where(attn_mask, -1e30, 0.0).astype(np.float32)[
            None, None
        ]
    if padding_mask.any():
        bias_eff = bias_eff + np.where(padding_mask, -1e30, 0.0).astype(np.float32)[
            :, None, None, :
        ]
    # [B, NH, t, s] so scores come out in transposed layout; exp() on host so
    # the kernel multiplies instead of adds (exp(-1e30) == 0 handles masks).
    expbT = np.exp(bias_eff.transpose(0, 1, 3, 2)).astype(BF16)
    # partition-interleave t: [B, NH, 128(p), TT, S]
    expb_r = np.ascontiguousarray(
        expbT.reshape(B, NUM_HEAD, TT, 128, S).transpose(0, 1, 3, 2, 4)
    )

    def relay_w(w):  # [H(k), 512(j)] -> [128(p), KO, NP, 128(j)]
        return np.ascontiguousarray(
            w.reshape(KO, 128, NP, 128).transpose(1, 0, 2, 3)
        )

    in_maps = []
    for c in range(N_CORES):
        bc, gc = divmod(c, 2)
        jsl = slice(gc * 512, (gc + 1) * 512)
        in_maps.append(
            {
                "xt": xt_full[bc],
                "wq": relay_w(wqT[:, jsl]),
                "wk": relay_w(wkT[:, jsl]),
                "wv": relay_w(wvT[:, jsl]),
                "expb": np.ascontiguousarray(
                    expb_r[bc, gc * 8 : (gc + 1) * 8]
                ),
            }
        )

    if "nc" not in _CACHE:
        _CACHE["nc"] = _build_bass()
    nc = _CACHE["nc"]

    res = run_bass_kernel_spmd(
        nc, in_maps, core_ids=list(range(N_CORES)), trace=TRACE
    )
    LAST_RESULTS = res

    full = np.empty((B, S, H), np.float32)
    for c in range(N_CORES):
        bc, gc = divmod(c, 2)
        oc = np.asarray(res.results[c]["out"]).astype(np.float32)
        oc = oc.reshape(NHEADS, 128, TT, HD + 1)
        num = oc[..., :HD]
        den = oc[..., HD]
        o = num / den[..., None]                        # [nh, p, sc, d]
        o = o.transpose(0, 2, 1, 3).reshape(NHEADS, S, HD)  # s = sc*128 + p
        full[bc, :, gc * 512 : (gc + 1) * 512] = (
            o.transpose(1, 0, 2).reshape(S, NHEADS * HD)
        )
    return full
